# revision 32
# baseline (speedup 1.0000x reference)
"""Complex-valued attention (magnitude-softmax with phase reconstruction) on 8 TRN2 cores.

Sharding: core c -> (batch b = c//2, head-group g = c%2). Each core projects
only its 8 heads (wqkv columns host-sliced per core), runs the attention loop
for those heads over ALL 1024 queries, and computes a partial output
contraction over its 512 e-columns of Wout. A pairwise AllReduce(add) over
{2b, 2b+1} then produces the full [N, D] output on both cores (assembled from
the even cores host-side). No token permutation is needed anywhere.

Math (per head h, scale S = 1/8):
  w = x @ Wqkv^T                         (bf16 matmuls, fp32 accum)
  z = dots^T[k, q]                       computed TRANSPOSED via stacked
      re/im contraction so softmax ends up on the partition axis
  m2 = z_re^2 + z_im^2                   (custom fused DVE op, bf16 inputs)
  l = ln(m2+eps);  e1 = exp(0.5*l + ln(S)) = S*mag   (affine fused into ACT)
  e = exp(e1)  (softmax numerator; mags are bounded so no max-subtraction)
  rm = 1/e1 (DVE reciprocal, in place);  f = e * rm;  attnU = z * f
  oh^T = sum_k wcombo^T @ attnU^T ; scaled by SCALE/sum_k e via ones=8.0
      S-matmul + reciprocal + Pool partition_broadcast
  out_partial = oh @ WoutT[my cols] ; AllReduce(add) over the core pair

Hermitian trick: z is Hermitian per head, so only the lower-triangle pair-
strips (k-tile pair p x padded cols [0, (2p+2)*128), 62.5% of cells) run the
dots matmuls + elementwise chain. Upper cells of areF/aimF/eeF are
reconstructed by XBAR DMA-transposing each row's strictly-lower-pair cols;
the aim sign flip (z antisymmetric imag) is folded into a negated stationary
(wc2n) on the transposed column ranges of the oh matmuls.

Engine balance (steady state):
  PE:   dots mm (lower strips) + oh mm + S mm (+ stage-1/stage-4 interleaved)
  ACT:  zre/zim PSUM->SBUF bf16 evictions, Ln (in place), Exp(e1), Exp(ee)
        (one table set: natural_log_exp_and_others, forced via patch)
  DVE:  sqmag, recip (in place), f-mul, attnU re/im muls (bf16 2x mode)
  XBAR: wc1 + 6 strip transposes per head (serial unit, co-paces the loop)
Tuned: ~574us HW exec (from 724us baseline measured on this harness).
"""

import os
import sys
import numpy as np
import ml_dtypes

sys.path.insert(0, "/opt/trn_rl_repo")

from contextlib import ExitStack

import concourse.bass as bass
import concourse.tile as tile
from concourse import bacc, mybir, dve_ops
from concourse.bass_utils import run_bass_kernel_spmd
from concourse.dve_spec import Spec, Src0, Src1, sq, lower, _has_src1
from concourse.dve_uop import DveOpSpec


def _register_sqmag():
    """Custom DVE op: out = Src0^2 + Src1^2 (one instr instead of 2 ACT + 1 DVE)."""
    name = "TENSOR_SQMAG_ANT"
    if name in dve_ops._SUB_OPCODE_FOR_NAME:
        return next(o for o in dve_ops.OPS if o.name == name)
    spec = Spec(
        body=sq(Src0) + sq(Src1),
        reference=lambda in0, in1, s0, s1, imm2:
            (in0.astype(np.float32) ** 2 + in1.astype(np.float32) ** 2),
    )
    opcode = max(dve_ops._SUB_OPCODE_FOR_NAME.values()) + 1
    dve_ops._SUB_OPCODE_FOR_NAME[name] = opcode
    shas = {}
    for ver in ("v3", "v4"):
        uops = lower(spec, ver=ver)
        shas[ver] = DveOpSpec(name=name, opcode=opcode, uops=uops,
                              rd1_en=_has_src1(spec)).sha(ver)
    op = dve_ops.DveOp(name, spec, subdim=False, uops_sha=shas)
    dve_ops.OPS.append(op)
    dve_ops.CUSTOM_DVE_SPECS[name] = spec
    return op


SQMAG = _register_sqmag()


def _patch_act_tables():
    """Force exp/ln to resolve to the combined natural_log_exp_and_others set
    so the per-tile Ln->Exp alternation doesn't reload ACT tables (~2.7us each)."""
    import concourse.bacc as _bacc
    if getattr(_bacc, "_act_tables_patched", False):
        return
    orig = _bacc.get_activation_tables
    AFT = mybir.ActivationFunctionType

    def patched(arch):
        t = {k: set(v) for k, v in orig(arch).items()}
        for name, fns in t.items():
            if name != "natural_log_exp_and_others":
                fns.discard(AFT.Exp)
                fns.discard(AFT.Ln)
        return t

    _bacc.get_activation_tables = patched
    _bacc._act_tables_patched = True


_patch_act_tables()

B, N, D, H, DH = 4, 1024, 1024, 16, 64
E = H * DH          # 1024
HL = 8              # heads per core
EL = 512            # local e-columns (HL * DH)
NQ = N              # queries per core: all 1024
KT = 8              # key tiles of 128
DT = 8              # d (contraction) tiles of 128
ETL = 4             # local e tiles of 128 (2 heads each)
SCALE = DH ** -0.5  # 0.125
LN_S = float(np.log(SCALE))
EPS = 1e-20

FP32 = mybir.dt.float32
BF16 = mybir.dt.bfloat16
AF = mybir.ActivationFunctionType
ALU = mybir.AluOpType

PAIRS = [[0, 1], [2, 3], [4, 5], [6, 7]]

_CACHE = {}


def build_graph():
    nc = bacc.Bacc("TRN2", target_bir_lowering=False, debug=False,
                   enable_asserts=False, num_devices=8)

    xTr_d = nc.declare_dram_parameter("xTr", [D, N], BF16, isOutput=False)
    xTi_d = nc.declare_dram_parameter("xTi", [D, N], BF16, isOutput=False)
    wqr_d = nc.declare_dram_parameter("wqTr", [D, EL], BF16, isOutput=False)   # my Wqkv_re.T cols
    wqi_d = nc.declare_dram_parameter("wqTi", [D, EL], BF16, isOutput=False)   # my Wqkv_im.T cols
    wqin_d = nc.declare_dram_parameter("wqTin", [D, EL], BF16, isOutput=False)  # -my Wqkv_im.T cols
    wo_d = nc.declare_dram_parameter("woS", [3 * EL, D], BF16, isOutput=False)  # [Wout_re.T; Wout_im.T; -Wout_im.T] my rows
    # outputs: per d-half chunk, rows = all N queries, cols = [re 512 | im 512]
    # bf16: halves the AllReduce + copy volume in the output tail
    o0_d = nc.declare_dram_parameter("o0", [N, 1024], BF16, isOutput=True)
    o1_d = nc.declare_dram_parameter("o1", [N, 1024], BF16, isOutput=True)

    with tile.TileContext(nc) as tc, ExitStack() as ctx:
        const_pool = ctx.enter_context(tc.tile_pool(name="const", bufs=1))
        xpool = ctx.enter_context(tc.tile_pool(name="x", bufs=1))
        wqpool = ctx.enter_context(tc.tile_pool(name="wq", bufs=1))
        apool = ctx.enter_context(tc.tile_pool(name="A", bufs=1))
        bpool = ctx.enter_context(tc.tile_pool(name="B", bufs=2))
        wcpool = ctx.enter_context(tc.tile_pool(name="wc", bufs=2))
        ohpool = ctx.enter_context(tc.tile_pool(name="oh", bufs=1))
        wopool = ctx.enter_context(tc.tile_pool(name="wo", bufs=1))
        epool = ctx.enter_context(tc.tile_pool(name="elem", bufs=2))
        spool = ctx.enter_context(tc.tile_pool(name="sm", bufs=2))
        opool = ctx.enter_context(tc.tile_pool(name="ostage", bufs=1))
        stkpool = ctx.enter_context(tc.tile_pool(name="stk", bufs=1))
        psd = ctx.enter_context(tc.tile_pool(name="psd", bufs=2, space="PSUM"))
        psd1 = ctx.enter_context(tc.tile_pool(name="psd1", bufs=1, space="PSUM"))
        psoh = ctx.enter_context(tc.tile_pool(name="psoh", bufs=1, space="PSUM"))
        psS = ctx.enter_context(tc.tile_pool(name="psS", bufs=2, space="PSUM"))
        drpool = ctx.enter_context(tc.tile_pool(name="dram", bufs=2, space="DRAM"))

        # ---- constants ----
        # ones = 8.0 folds the SCALE factor into the softmax-sum reciprocal:
        # rs = 1/(8*sum e) = SCALE/sum e
        ones_bf = const_pool.tile([128, 1], BF16, tag="ones")
        nc.gpsimd.memset(ones_bf[:], 1.0 / SCALE)
        eps_t = const_pool.tile([128, 1], FP32, tag="eps")
        nc.gpsimd.memset(eps_t[:], EPS)
        lnS_t = const_pool.tile([128, 1], FP32, tag="lnS")
        nc.gpsimd.memset(lnS_t[:], LN_S)
        nlnS_t = const_pool.tile([128, 1], FP32, tag="nlnS")
        nc.gpsimd.memset(nlnS_t[:], -LN_S)

        # ---- resident loads ----
        # x^T as [128, DT, N] (partition = d within tile)
        xr = xpool.tile([128, DT, N], BF16, tag="xr")
        xi = xpool.tile([128, DT, N], BF16, tag="xi")
        nc.sync.dma_start(out=xr[:], in_=xTr_d.ap().rearrange("(t p) n -> p t n", p=128))
        nc.sync.dma_start(out=xi[:], in_=xTi_d.ap().rearrange("(t p) n -> p t n", p=128))
        # A stacks: per head [128 (wr 0:64 | wi 64:128), N]; ring of 6 slots
        A = []

        # ---- stage 1: w^T = Wqkv~ @ x^T (my 8 heads), evicted to per-head stacks ----
        def stage1_et(et):
            A.append(apool.tile([128, N], BF16, tag="A", name=f"A{2*et}", bufs=4))
            A.append(apool.tile([128, N], BF16, tag="A", name=f"A{2*et+1}", bufs=4))
            wslab_r = wqpool.tile([128, DT, 128], BF16, tag="wslab_r")
            wslab_i = wqpool.tile([128, DT, 128], BF16, tag="wslab_i")
            wslab_in = wqpool.tile([128, DT, 128], BF16, tag="wslab_in")
            esl = slice(et * 128, (et + 1) * 128)
            nc.sync.dma_start(out=wslab_r[:], in_=wqr_d.ap()[:, esl].rearrange("(t p) n -> p t n", p=128))
            nc.sync.dma_start(out=wslab_i[:], in_=wqi_d.ap()[:, esl].rearrange("(t p) n -> p t n", p=128))
            nc.sync.dma_start(out=wslab_in[:], in_=wqin_d.ap()[:, esl].rearrange("(t p) n -> p t n", p=128))
            h0, h1 = 2 * et, 2 * et + 1
            Ar = A[h0].rearrange("p (c n) -> p c n", c=2)
            Ai = A[h1].rearrange("p (c n) -> p c n", c=2)
            # per-nch PSUM tiles: holds only 2 banks at a time (vs 4) so the
            # head-loop dots keep a double-buffer, and heads can start on the
            # first token half before the second is projected
            for nch in range(2):
                nsl = slice(nch * 512, (nch + 1) * 512)
                ps_re = psd1.tile([128, 512], FP32, tag="s1")
                ps_im = psd1.tile([128, 512], FP32, tag="s1")
                for dt_ in range(DT):
                    first, last = dt_ == 0, dt_ == DT - 1
                    # w_re += Wr^T x_r ; w_re += (-Wi^T) x_i
                    nc.tensor.matmul(ps_re[:], wslab_r[:, dt_, :], xr[:, dt_, nsl],
                                     start=first, stop=False)
                    nc.tensor.matmul(ps_re[:], wslab_in[:, dt_, :], xi[:, dt_, nsl],
                                     start=False, stop=last)
                    # w_im += Wi^T x_r ; w_im += Wr^T x_i
                    nc.tensor.matmul(ps_im[:], wslab_i[:, dt_, :], xr[:, dt_, nsl],
                                     start=first, stop=False)
                    nc.tensor.matmul(ps_im[:], wslab_r[:, dt_, :], xi[:, dt_, nsl],
                                     start=False, stop=last)
                # evict into head stacks (cast to bf16)
                nc.scalar.copy(Ar[0:64, nch, :], ps_re[0:64, :])
                nc.scalar.copy(Ai[0:64, nch, :], ps_re[64:128, :])
                nc.scalar.copy(Ar[64:128, nch, :], ps_im[0:64, :])
                nc.scalar.copy(Ai[64:128, nch, :], ps_im[64:128, :])

        # ---- per-head attention (Hermitian: only lower-triangle pair-strips) ----
        # oh^T stacks for stage 4: [128, ETL, NQ] bf16 (ohin folded into wos)
        ohr = ohpool.tile([128, ETL, NQ], BF16, tag="ohr")
        ohi = ohpool.tile([128, ETL, NQ], BF16, tag="ohi")

        # pair p covers k-tiles {2p, 2p+1} and padded q-cols [0, (2p+2)*128),
        # processed in <=512-wide chunks; the upper triangle is reconstructed by
        # DMA-transposing each row's strictly-lower-pair cols [0, 2p*128).
        PAIR_CHUNKS = [
            [(0, 256)],
            [(0, 512)],
            [(0, 512), (512, 256)],
            [(0, 512), (512, 512)],
        ]

        def head(h):
            Ah = A[h]
            # B_h = [-wi; wr]; Bhn = -B_h (stationary for transposed-aim pieces)
            Bh = bpool.tile([128, N], BF16, tag="B", bufs=1)
            nc.vector.tensor_scalar_mul(Bh[0:64, :], Ah[64:128, :], -1.0)
            nc.vector.tensor_copy(Bh[64:128, :], Ah[0:64, :])
            Bhn = bpool.tile([128, N], BF16, tag="Bn", bufs=1)
            nc.vector.tensor_scalar_mul(Bhn[:], Bh[:], -1.0)

            wc1 = wcpool.tile([128, KT, 128], BF16, tag="wc1")
            wc2 = wcpool.tile([128, KT, 128], BF16, tag="wc2")
            wc2n = wcpool.tile([128, KT, 128], BF16, tag="wc2n")
            nc.sync.dma_start(wc1[:], Ah[:], transpose=True)
            nc.sync.dma_start(wc2[:], Bh[:], transpose=True)
            nc.sync.dma_start(wc2n[:], Bhn[:], transpose=True)

            areF = stkpool.tile([128, KT, N], BF16, tag="areF", bufs=2)
            aimF = stkpool.tile([128, KT, N], BF16, tag="aimF", bufs=2)
            eeF = stkpool.tile([128, KT, N], BF16, tag="eeF", bufs=1)

            # pairs in reverse order: the widest strip-transposes issue first,
            # so the XBAR drains while later (narrower) pairs compute and the
            # end-of-head S/oh phase no longer waits on transposes. Writes stay
            # disjoint: pair p's transposes target column-tiles >= 2p+2 while
            # pairs p' < p compute columns < (2p'+2)*128.
            for p in (3, 2, 1, 0):
                r2 = slice(2 * p, 2 * p + 2)
                for (qoff, w) in PAIR_CHUNKS[p]:
                    qws = slice(qoff, qoff + w)
                    zre = psd.tile([128, 2, 512], FP32, tag="dots")
                    zim = psd.tile([128, 2, 512], FP32, tag="dots")
                    # dots^T[k, q]: re = [wr;wi]_k . [wr;wi]_q ; im = [-wi;wr]_k . [wr;wi]_q
                    for i in range(2):
                        ksl = slice((2 * p + i) * 128, (2 * p + i + 1) * 128)
                        nc.tensor.matmul(zre[:, i, 0:w], Ah[:, ksl], Ah[:, qws],
                                         start=True, stop=True)
                        nc.tensor.matmul(zim[:, i, 0:w], Bh[:, ksl], Ah[:, qws],
                                         start=True, stop=True)

                    # evict both dots tensors as bf16: PSUM recycles fast and the
                    # tail multiplies run in DVE 2x mode
                    zreS = epool.tile([128, 2, 512], BF16, tag="zreS", bufs=2)
                    nc.scalar.copy(zreS[:, :, 0:w], zre[:, :, 0:w])
                    zimS = epool.tile([128, 2, 512], BF16, tag="zimS", bufs=2)
                    nc.scalar.copy(zimS[:, :, 0:w], zim[:, :, 0:w])
                    m2 = epool.tile([128, 2, 512], FP32, tag="m2", bufs=2)
                    nc.vector._custom_dve(SQMAG, out=m2[:, :, 0:w],
                                          in0=zreS[:, :, 0:w], in1=zimS[:, :, 0:w])
                    # ll = ln(m2+eps) computed in place over m2 (saves a tag)
                    nc.scalar.activation(m2[:, :, 0:w], m2[:, :, 0:w], AF.Ln, bias=eps_t[:])
                    # e1 = S*mag = exp(0.5*ln(m2) + ln(S)) -- affine fused into the ACT op
                    e1 = epool.tile([128, 2, 512], FP32, tag="e1", bufs=2)
                    nc.scalar.activation(e1[:, :, 0:w], m2[:, :, 0:w], AF.Exp,
                                         scale=0.5, bias=lnS_t[:])
                    nc.scalar.activation(eeF[:, r2, qws], e1[:, :, 0:w], AF.Exp)
                    # rm = 1/e1 in place over e1
                    nc.vector.reciprocal_approx_fast(out=e1[:, :, 0:w], in_=e1[:, :, 0:w])
                    ff = epool.tile([128, 2, 512], BF16, tag="ff", bufs=1)
                    nc.vector.tensor_mul(ff[:, :, 0:w], eeF[:, r2, qws], e1[:, :, 0:w])
                    nc.vector.tensor_mul(areF[:, r2, qws], zreS[:, :, 0:w], ff[:, :, 0:w])
                    nc.vector.tensor_mul(aimF[:, r2, qws], zimS[:, :, 0:w], ff[:, :, 0:w])

                # reconstruct upper blocks: transpose each row's strictly-lower
                # pair cols [0, 2p*128) into column-tile kt of rows 0..2p-1
                if p >= 1:
                    src_cols = slice(0, 2 * p * 128)
                    for i in range(2):
                        kt = 2 * p + i
                        ktsl = slice(kt * 128, (kt + 1) * 128)
                        for F in (areF, aimF, eeF):
                            nc.sync.dma_start(F[:, 0:2 * p, ktsl], F[:, kt, src_cols],
                                              transpose=True)

            et2, half = h // 2, (h % 2) * 64
            hs = slice(half, half + 64)

            # softmax-sum matmuls first (releases eeF for the next head early)
            ps_s = []
            for qch in range(2):
                qsl = slice(qch * 512, (qch + 1) * 512)
                ps = psS.tile([1, 512], FP32, tag="S")
                for kt in range(KT):
                    nc.tensor.matmul(ps[:], ones_bf[:], eeF[:, kt, qsl],
                                     start=(kt == 0), stop=(kt == KT - 1))
                ps_s.append(ps)

            for qch in range(2):
                qoff = qch * 512
                qsl = slice(qoff, qoff + 512)
                ps_oh = psoh.tile([128, 512], FP32, tag="oh")
                for kt in range(KT):
                    first, last = kt == 0, kt == KT - 1
                    nc.tensor.matmul(ps_oh[:], wc1[:, kt, :], areF[:, kt, qsl],
                                     start=first, stop=False)
                    # aim sign: cols < pair-width are computed (true values, wc2);
                    # cols >= pair-width are transposed (negate via wc2n)
                    b = (2 * (kt // 2) + 2) * 128 - qoff
                    lo = max(0, min(512, b))
                    if lo > 0:
                        nc.tensor.matmul(ps_oh[:, 0:lo], wc2[:, kt, :],
                                         aimF[:, kt, qoff:qoff + lo],
                                         start=False, stop=last)
                    if lo < 512:
                        nc.tensor.matmul(ps_oh[:, lo:512], wc2n[:, kt, :],
                                         aimF[:, kt, qoff + lo:qoff + 512],
                                         start=False, stop=last)

                # evict raw oh^T immediately (releases psoh for the next chunk)
                ohraw = spool.tile([128, 512], FP32, tag="ohraw", bufs=2)
                nc.scalar.copy(ohraw[:], ps_oh[:])
                # rs = SCALE/sum(e) (ones=8 folded); broadcast via Pool
                rs = spool.tile([1, 512], FP32, tag="rs", bufs=1)
                nc.vector.reciprocal_approx_fast(out=rs[:], in_=ps_s[qch][:])
                bbS = spool.tile([128, 512], FP32, tag="bbS", bufs=1)
                nc.gpsimd.partition_broadcast(bbS[:], rs[:], channels=128)

                nc.vector.tensor_mul(ohr[hs, et2, qsl], ohraw[0:64, :], bbS[0:64, :])
                nc.vector.tensor_mul(ohi[hs, et2, qsl], ohraw[64:128, :], bbS[64:128, :])

        # software-pipelined drive: keep stage 1 ~1 e-tile ahead of heads
        stage1_et(0)
        stage1_et(1)
        head(0); head(1)
        stage1_et(2)
        head(2); head(3)
        stage1_et(3)
        head(4); head(5); head(6); head(7)

        # ---- stage 4: out_partial = oh @ WoutT[my cols]; AllReduce over pair ----
        for dc in range(2):
            dsl = slice(dc * 512, (dc + 1) * 512)
            wos = wopool.tile([128, 3 * ETL, 512], BF16, tag="wos")
            nc.sync.dma_start(out=wos[:], in_=wo_d.ap()[:, dsl].rearrange("(t p) n -> p t n", p=128))
            ob = drpool.tile([N, 1024], BF16, tag="ob")
            obv = ob.rearrange("n (c d) -> n c d", c=2)
            for qt in range(8):
                qsl = slice(qt * 128, (qt + 1) * 128)
                po = psd.tile([128, 2, 512], FP32, tag="dots")
                for et in range(ETL):
                    first = et == 0
                    # out_re = ohr.wr + ohi.(-wi) ; out_im = ohi.wr + ohr.wi
                    nc.tensor.matmul(po[:, 0, :], ohr[:, et, qsl], wos[:, et, :],
                                     start=first, stop=False)
                    nc.tensor.matmul(po[:, 0, :], ohi[:, et, qsl], wos[:, 2 * ETL + et, :],
                                     start=False, stop=(et == ETL - 1))
                    nc.tensor.matmul(po[:, 1, :], ohi[:, et, qsl], wos[:, et, :],
                                     start=first, stop=False)
                    nc.tensor.matmul(po[:, 1, :], ohr[:, et, qsl], wos[:, ETL + et, :],
                                     start=False, stop=(et == ETL - 1))
                o_st = opool.tile([128, 2, 512], BF16, tag="ost", bufs=1)
                nc.scalar.copy(o_st[:], po[:])
                nc.sync.dma_start(out=obv[qsl, :, :], in_=o_st[:])
            out_d = o0_d if dc == 0 else o1_d
            obr = drpool.tile([N, 1024], BF16, tag="obr")
            nc.gpsimd.collective_compute(
                "AllReduce", ALU.add, replica_groups=PAIRS,
                ins=[ob.opt()], outs=[obr.opt()],
            )
            nc.sync.dma_start(out=out_d.ap(), in_=obr[:])

    nc.compile()
    return nc


def _to_bf16(a):
    return np.asarray(a, dtype=np.float32).astype(ml_dtypes.bfloat16)


def make_in_maps(x_re, x_im, wqkv_re, wqkv_im, wout_re, wout_im, bout_re, bout_im):
    x_re = np.asarray(x_re, np.float32)
    x_im = np.asarray(x_im, np.float32)
    wqT_r = np.asarray(wqkv_re, np.float32).T   # [D, E]
    wqT_i = np.asarray(wqkv_im, np.float32).T
    woT_r = np.asarray(wout_re, np.float32).T   # [E, D]
    woT_i = np.asarray(wout_im, np.float32).T

    in_maps = []
    for c in range(8):
        b, g = c // 2, c % 2
        esl = slice(g * EL, (g + 1) * EL)
        in_maps.append({
            "xTr": _to_bf16(np.ascontiguousarray(x_re[b].T)),
            "xTi": _to_bf16(np.ascontiguousarray(x_im[b].T)),
            "wqTr": _to_bf16(np.ascontiguousarray(wqT_r[:, esl])),
            "wqTi": _to_bf16(np.ascontiguousarray(wqT_i[:, esl])),
            "wqTin": _to_bf16(np.ascontiguousarray(-wqT_i[:, esl])),
            "woS": _to_bf16(np.concatenate([woT_r[esl, :], woT_i[esl, :], -woT_i[esl, :]], axis=0)),
        })
    return in_maps


def assemble_output(res, bout_re, bout_im):
    out = np.zeros((B, N, D), np.complex64)
    for b in range(B):
        o0 = np.asarray(res[2 * b]["o0"], np.float32)
        o1 = np.asarray(res[2 * b]["o1"], np.float32)
        out[b, :, 0:512] = o0[:, 0:512] + 1j * o0[:, 512:1024]
        out[b, :, 512:1024] = o1[:, 0:512] + 1j * o1[:, 512:1024]

    # bout is zero in this problem; add anyway for faithfulness
    out += (np.asarray(bout_re, np.float32) + 1j * np.asarray(bout_im, np.float32))[None, None, :]
    return out


def kernel(**inputs):
    if "nc" not in _CACHE:
        _CACHE["nc"] = build_graph()
    nc = _CACHE["nc"]
    in_maps = make_in_maps(**inputs)
    res = run_bass_kernel_spmd(nc, in_maps, core_ids=list(range(8))).results
    return assemble_output(res, inputs["bout_re"], inputs["bout_im"])


# revision 33
# speedup vs baseline: 1.1266x; 1.1266x over previous
"""Complex-valued attention (magnitude-softmax with phase reconstruction) on 8 TRN2 cores.

Sharding: core c -> (batch b = c//2, head-group g = c%2). Each core projects
only its 8 heads (wqkv columns host-sliced per core), runs the attention loop
for those heads over ALL 1024 queries, and computes a partial output
contraction over its 512 e-columns of Wout. A pairwise AllReduce(add) over
{2b, 2b+1} then produces the full [N, D] output on both cores (assembled from
the even cores host-side). No token permutation is needed anywhere.

Math (per head h, scale S = 1/8):
  w = x @ Wqkv^T                         (bf16 matmuls, fp32 accum)
  z = dots^T[k, q]                       computed TRANSPOSED via stacked
      re/im contraction so softmax ends up on the partition axis
  m2 = z_re^2 + z_im^2                   (custom fused DVE op, bf16 inputs)
  l = ln(m2+eps);  e1 = exp(0.5*l + ln(S)) = S*mag   (affine fused into ACT)
  e = exp(e1)  (softmax numerator; mags are bounded so no max-subtraction)
  rm = exp(-0.5*l - ln(S)) = 1/(S*mag)   (ACT, same table)
  f = e * rm;  attnU = z * f  (= 8 * e * unit_phase)
  oh^T = sum_k wcombo^T @ attnU^T ; scaled by SCALE/sum_k e via ones=8.0
      S-matmul + reciprocal + Pool partition_broadcast
  out_partial = oh @ WoutT[my cols] ; AllReduce(add) over the core pair

Engine balance per kp (2 key tiles x 512 queries):
  PE:   4 dots mm + 4 oh mm + 2 S mm (+ stage-1/stage-4 interleaved)
  Pool: zre/zim PSUM->SBUF bf16 evictions + per-(h,qch) partition_broadcast
  ACT:  Ln, Exp(e1), Exp(ee), Exp(rm) (one table set: natural_log_exp_and_others)
  DVE:  sqmag, f-mul, attnU re/im muls (bf16 2x mode)
"""

import os
import sys
import numpy as np
import ml_dtypes

sys.path.insert(0, "/opt/trn_rl_repo")

from contextlib import ExitStack

import concourse.bass as bass
import concourse.tile as tile
from concourse import bacc, mybir, dve_ops
from concourse.bass_utils import run_bass_kernel_spmd
from concourse.dve_spec import Spec, Src0, Src1, sq, lower, _has_src1
from concourse.dve_uop import DveOpSpec


def _register_sqmag():
    """Custom DVE op: out = Src0^2 + Src1^2 (one instr instead of 2 ACT + 1 DVE)."""
    name = "TENSOR_SQMAG_ANT"
    if name in dve_ops._SUB_OPCODE_FOR_NAME:
        return next(o for o in dve_ops.OPS if o.name == name)
    spec = Spec(
        body=sq(Src0) + sq(Src1),
        reference=lambda in0, in1, s0, s1, imm2:
            (in0.astype(np.float32) ** 2 + in1.astype(np.float32) ** 2),
    )
    opcode = max(dve_ops._SUB_OPCODE_FOR_NAME.values()) + 1
    dve_ops._SUB_OPCODE_FOR_NAME[name] = opcode
    shas = {}
    for ver in ("v3", "v4"):
        uops = lower(spec, ver=ver)
        shas[ver] = DveOpSpec(name=name, opcode=opcode, uops=uops,
                              rd1_en=_has_src1(spec)).sha(ver)
    op = dve_ops.DveOp(name, spec, subdim=False, uops_sha=shas)
    dve_ops.OPS.append(op)
    dve_ops.CUSTOM_DVE_SPECS[name] = spec
    return op


SQMAG = _register_sqmag()


def _patch_act_tables():
    """Force exp/ln to resolve to the combined natural_log_exp_and_others set
    so the per-tile Ln->Exp alternation doesn't reload ACT tables (~2.7us each)."""
    import concourse.bacc as _bacc
    if getattr(_bacc, "_act_tables_patched", False):
        return
    orig = _bacc.get_activation_tables
    AFT = mybir.ActivationFunctionType

    def patched(arch):
        t = {k: set(v) for k, v in orig(arch).items()}
        for name, fns in t.items():
            if name != "natural_log_exp_and_others":
                fns.discard(AFT.Exp)
                fns.discard(AFT.Ln)
        return t

    _bacc.get_activation_tables = patched
    _bacc._act_tables_patched = True


_patch_act_tables()

B, N, D, H, DH = 4, 1024, 1024, 16, 64
E = H * DH          # 1024
HL = 8              # heads per core
EL = 512            # local e-columns (HL * DH)
NQ = N              # queries per core: all 1024
KT = 8              # key tiles of 128
DT = 8              # d (contraction) tiles of 128
ETL = 4             # local e tiles of 128 (2 heads each)
SCALE = DH ** -0.5  # 0.125
LN_S = float(np.log(SCALE))
EPS = 1e-20

FP32 = mybir.dt.float32
BF16 = mybir.dt.bfloat16
AF = mybir.ActivationFunctionType
ALU = mybir.AluOpType

PAIRS = [[0, 1], [2, 3], [4, 5], [6, 7]]

_CACHE = {}


def build_graph():
    nc = bacc.Bacc("TRN2", target_bir_lowering=False, debug=False,
                   enable_asserts=False, num_devices=8)

    xTr_d = nc.declare_dram_parameter("xTr", [D, N], BF16, isOutput=False)
    xTi_d = nc.declare_dram_parameter("xTi", [D, N], BF16, isOutput=False)
    wqr_d = nc.declare_dram_parameter("wqTr", [D, EL], BF16, isOutput=False)   # my Wqkv_re.T cols
    wqi_d = nc.declare_dram_parameter("wqTi", [D, EL], BF16, isOutput=False)   # my Wqkv_im.T cols
    wqin_d = nc.declare_dram_parameter("wqTin", [D, EL], BF16, isOutput=False)  # -my Wqkv_im.T cols
    wo_d = nc.declare_dram_parameter("woS", [3 * EL, D], BF16, isOutput=False)  # [Wout_re.T; Wout_im.T; -Wout_im.T] my rows
    # outputs: per d-half chunk, rows = all N queries, cols = [re 512 | im 512]
    # bf16: halves the AllReduce + copy volume in the output tail
    o0_d = nc.declare_dram_parameter("o0", [N, 1024], BF16, isOutput=True)
    o1_d = nc.declare_dram_parameter("o1", [N, 1024], BF16, isOutput=True)

    with tile.TileContext(nc) as tc, ExitStack() as ctx:
        const_pool = ctx.enter_context(tc.tile_pool(name="const", bufs=1))
        xpool = ctx.enter_context(tc.tile_pool(name="x", bufs=1))
        wqpool = ctx.enter_context(tc.tile_pool(name="wq", bufs=1))
        apool = ctx.enter_context(tc.tile_pool(name="A", bufs=1))
        bpool = ctx.enter_context(tc.tile_pool(name="B", bufs=2))
        wcpool = ctx.enter_context(tc.tile_pool(name="wc", bufs=2))
        ohpool = ctx.enter_context(tc.tile_pool(name="oh", bufs=1))
        wopool = ctx.enter_context(tc.tile_pool(name="wo", bufs=1))
        epool = ctx.enter_context(tc.tile_pool(name="elem", bufs=2))
        spool = ctx.enter_context(tc.tile_pool(name="sm", bufs=2))
        opool = ctx.enter_context(tc.tile_pool(name="ostage", bufs=1))
        stkpool = ctx.enter_context(tc.tile_pool(name="stk", bufs=1))
        psd = ctx.enter_context(tc.tile_pool(name="psd", bufs=2, space="PSUM"))
        psd1 = ctx.enter_context(tc.tile_pool(name="psd1", bufs=1, space="PSUM"))
        psoh = ctx.enter_context(tc.tile_pool(name="psoh", bufs=1, space="PSUM"))
        psS = ctx.enter_context(tc.tile_pool(name="psS", bufs=2, space="PSUM"))
        drpool = ctx.enter_context(tc.tile_pool(name="dram", bufs=2, space="DRAM"))

        # ---- constants ----
        # ones = 8.0 folds the SCALE factor into the softmax-sum reciprocal:
        # rs = 1/(8*sum e) = SCALE/sum e
        ones_bf = const_pool.tile([128, 1], BF16, tag="ones")
        nc.gpsimd.memset(ones_bf[:], 1.0 / SCALE)
        eps_t = const_pool.tile([128, 1], FP32, tag="eps")
        nc.gpsimd.memset(eps_t[:], EPS)
        lnS_t = const_pool.tile([128, 1], FP32, tag="lnS")
        nc.gpsimd.memset(lnS_t[:], LN_S)
        nlnS_t = const_pool.tile([128, 1], FP32, tag="nlnS")
        nc.gpsimd.memset(nlnS_t[:], -LN_S)

        # ---- resident loads ----
        # x^T as [128, DT, N] (partition = d within tile)
        xr = xpool.tile([128, DT, N], BF16, tag="xr")
        xi = xpool.tile([128, DT, N], BF16, tag="xi")
        nc.sync.dma_start(out=xr[:], in_=xTr_d.ap().rearrange("(t p) n -> p t n", p=128))
        nc.sync.dma_start(out=xi[:], in_=xTi_d.ap().rearrange("(t p) n -> p t n", p=128))
        # A stacks: per head [128 (wr 0:64 | wi 64:128), N]; ring of 6 slots
        A = []

        # ---- stage 1: w^T = Wqkv~ @ x^T (my 8 heads), evicted to per-head stacks ----
        def stage1_et(et):
            A.append(apool.tile([128, N], BF16, tag="A", name=f"A{2*et}", bufs=4))
            A.append(apool.tile([128, N], BF16, tag="A", name=f"A{2*et+1}", bufs=4))
            wslab_r = wqpool.tile([128, DT, 128], BF16, tag="wslab_r")
            wslab_i = wqpool.tile([128, DT, 128], BF16, tag="wslab_i")
            wslab_in = wqpool.tile([128, DT, 128], BF16, tag="wslab_in")
            esl = slice(et * 128, (et + 1) * 128)
            nc.sync.dma_start(out=wslab_r[:], in_=wqr_d.ap()[:, esl].rearrange("(t p) n -> p t n", p=128))
            nc.sync.dma_start(out=wslab_i[:], in_=wqi_d.ap()[:, esl].rearrange("(t p) n -> p t n", p=128))
            nc.sync.dma_start(out=wslab_in[:], in_=wqin_d.ap()[:, esl].rearrange("(t p) n -> p t n", p=128))
            h0, h1 = 2 * et, 2 * et + 1
            Ar = A[h0].rearrange("p (c n) -> p c n", c=2)
            Ai = A[h1].rearrange("p (c n) -> p c n", c=2)
            # per-nch PSUM tiles: holds only 2 banks at a time (vs 4) so the
            # head-loop dots keep a double-buffer, and heads can start on the
            # first token half before the second is projected
            for nch in range(2):
                nsl = slice(nch * 512, (nch + 1) * 512)
                ps_re = psd1.tile([128, 512], FP32, tag="s1")
                ps_im = psd1.tile([128, 512], FP32, tag="s1")
                for dt_ in range(DT):
                    first, last = dt_ == 0, dt_ == DT - 1
                    # w_re += Wr^T x_r ; w_re += (-Wi^T) x_i
                    nc.tensor.matmul(ps_re[:], wslab_r[:, dt_, :], xr[:, dt_, nsl],
                                     start=first, stop=False)
                    nc.tensor.matmul(ps_re[:], wslab_in[:, dt_, :], xi[:, dt_, nsl],
                                     start=False, stop=last)
                    # w_im += Wi^T x_r ; w_im += Wr^T x_i
                    nc.tensor.matmul(ps_im[:], wslab_i[:, dt_, :], xr[:, dt_, nsl],
                                     start=first, stop=False)
                    nc.tensor.matmul(ps_im[:], wslab_r[:, dt_, :], xi[:, dt_, nsl],
                                     start=False, stop=last)
                # evict into head stacks (cast to bf16)
                nc.scalar.copy(Ar[0:64, nch, :], ps_re[0:64, :])
                nc.scalar.copy(Ai[0:64, nch, :], ps_re[64:128, :])
                nc.scalar.copy(Ar[64:128, nch, :], ps_im[0:64, :])
                nc.scalar.copy(Ai[64:128, nch, :], ps_im[64:128, :])

        # ---- per-head attention (Hermitian: only lower-triangle pair-strips) ----
        # oh^T stacks for stage 4: [128, ETL, NQ] bf16 (ohin folded into wos)
        ohr = ohpool.tile([128, ETL, NQ], BF16, tag="ohr")
        ohi = ohpool.tile([128, ETL, NQ], BF16, tag="ohi")

        # pair p covers k-tiles {2p, 2p+1} and padded q-cols [0, (2p+2)*128),
        # processed in <=512-wide chunks; the upper triangle is reconstructed by
        # DMA-transposing each row's strictly-lower-pair cols [0, 2p*128).
        PAIR_CHUNKS = [
            [(0, 256)],
            [(0, 512)],
            [(0, 512), (512, 256)],
            [(0, 512), (512, 512)],
        ]

        def head(h):
            Ah = A[h]
            # B_h = [-wi; wr]; Bhn = -B_h (stationary for transposed-aim pieces)
            Bh = bpool.tile([128, N], BF16, tag="B", bufs=1)
            nc.vector.tensor_scalar_mul(Bh[0:64, :], Ah[64:128, :], -1.0)
            nc.vector.tensor_copy(Bh[64:128, :], Ah[0:64, :])
            Bhn = bpool.tile([128, N], BF16, tag="Bn", bufs=1)
            nc.vector.tensor_scalar_mul(Bhn[:], Bh[:], -1.0)

            wc1 = wcpool.tile([128, KT, 128], BF16, tag="wc1")
            wc2 = wcpool.tile([128, KT, 128], BF16, tag="wc2")
            wc2n = wcpool.tile([128, KT, 128], BF16, tag="wc2n")
            nc.sync.dma_start(wc1[:], Ah[:], transpose=True)
            nc.sync.dma_start(wc2[:], Bh[:], transpose=True)
            nc.sync.dma_start(wc2n[:], Bhn[:], transpose=True)

            areF = stkpool.tile([128, KT, N], BF16, tag="areF", bufs=2)
            aimF = stkpool.tile([128, KT, N], BF16, tag="aimF", bufs=2)
            eeF = stkpool.tile([128, KT, N], BF16, tag="eeF", bufs=1)

            for p in range(4):
                r2 = slice(2 * p, 2 * p + 2)
                for (qoff, w) in PAIR_CHUNKS[p]:
                    qws = slice(qoff, qoff + w)
                    zre = psd.tile([128, 2, 512], FP32, tag="dots")
                    zim = psd.tile([128, 2, 512], FP32, tag="dots")
                    # dots^T[k, q]: re = [wr;wi]_k . [wr;wi]_q ; im = [-wi;wr]_k . [wr;wi]_q
                    for i in range(2):
                        ksl = slice((2 * p + i) * 128, (2 * p + i + 1) * 128)
                        nc.tensor.matmul(zre[:, i, 0:w], Ah[:, ksl], Ah[:, qws],
                                         start=True, stop=True)
                        nc.tensor.matmul(zim[:, i, 0:w], Bh[:, ksl], Ah[:, qws],
                                         start=True, stop=True)

                    # evict both dots tensors as bf16: PSUM recycles fast and the
                    # tail multiplies run in DVE 2x mode
                    zreS = epool.tile([128, 2, 512], BF16, tag="zreS", bufs=2)
                    nc.scalar.copy(zreS[:, :, 0:w], zre[:, :, 0:w])
                    zimS = epool.tile([128, 2, 512], BF16, tag="zimS", bufs=2)
                    nc.scalar.copy(zimS[:, :, 0:w], zim[:, :, 0:w])
                    m2 = epool.tile([128, 2, 512], FP32, tag="m2", bufs=2)
                    nc.vector._custom_dve(SQMAG, out=m2[:, :, 0:w],
                                          in0=zreS[:, :, 0:w], in1=zimS[:, :, 0:w])
                    # ll = ln(m2+eps) computed in place over m2 (saves a tag)
                    nc.scalar.activation(m2[:, :, 0:w], m2[:, :, 0:w], AF.Ln, bias=eps_t[:])
                    # e1 = S*mag = exp(0.5*ln(m2) + ln(S)) -- affine fused into the ACT op
                    e1 = epool.tile([128, 2, 512], FP32, tag="e1", bufs=2)
                    nc.scalar.activation(e1[:, :, 0:w], m2[:, :, 0:w], AF.Exp,
                                         scale=0.5, bias=lnS_t[:])
                    nc.scalar.activation(eeF[:, r2, qws], e1[:, :, 0:w], AF.Exp)
                    # rm = 1/e1 in place over e1
                    nc.vector.reciprocal_approx_fast(out=e1[:, :, 0:w], in_=e1[:, :, 0:w])
                    ff = epool.tile([128, 2, 512], BF16, tag="ff", bufs=1)
                    nc.vector.tensor_mul(ff[:, :, 0:w], eeF[:, r2, qws], e1[:, :, 0:w])
                    nc.vector.tensor_mul(areF[:, r2, qws], zreS[:, :, 0:w], ff[:, :, 0:w])
                    nc.vector.tensor_mul(aimF[:, r2, qws], zimS[:, :, 0:w], ff[:, :, 0:w])

                # reconstruct upper blocks: transpose each row's strictly-lower
                # pair cols [0, 2p*128) into column-tile kt of rows 0..2p-1
                if p >= 1:
                    src_cols = slice(0, 2 * p * 128)
                    for i in range(2):
                        kt = 2 * p + i
                        ktsl = slice(kt * 128, (kt + 1) * 128)
                        for F in (areF, aimF, eeF):
                            nc.sync.dma_start(F[:, 0:2 * p, ktsl], F[:, kt, src_cols],
                                              transpose=True)

            et2, half = h // 2, (h % 2) * 64
            hs = slice(half, half + 64)

            # softmax-sum matmuls first (releases eeF for the next head early)
            ps_s = []
            for qch in range(2):
                qsl = slice(qch * 512, (qch + 1) * 512)
                ps = psS.tile([1, 512], FP32, tag="S")
                for kt in range(KT):
                    nc.tensor.matmul(ps[:], ones_bf[:], eeF[:, kt, qsl],
                                     start=(kt == 0), stop=(kt == KT - 1))
                ps_s.append(ps)

            for qch in range(2):
                qoff = qch * 512
                qsl = slice(qoff, qoff + 512)
                ps_oh = psoh.tile([128, 512], FP32, tag="oh")
                for kt in range(KT):
                    first, last = kt == 0, kt == KT - 1
                    nc.tensor.matmul(ps_oh[:], wc1[:, kt, :], areF[:, kt, qsl],
                                     start=first, stop=False)
                    # aim sign: cols < pair-width are computed (true values, wc2);
                    # cols >= pair-width are transposed (negate via wc2n)
                    b = (2 * (kt // 2) + 2) * 128 - qoff
                    lo = max(0, min(512, b))
                    if lo > 0:
                        nc.tensor.matmul(ps_oh[:, 0:lo], wc2[:, kt, :],
                                         aimF[:, kt, qoff:qoff + lo],
                                         start=False, stop=last)
                    if lo < 512:
                        nc.tensor.matmul(ps_oh[:, lo:512], wc2n[:, kt, :],
                                         aimF[:, kt, qoff + lo:qoff + 512],
                                         start=False, stop=last)

                # evict raw oh^T immediately (releases psoh for the next chunk)
                ohraw = spool.tile([128, 512], FP32, tag="ohraw", bufs=2)
                nc.scalar.copy(ohraw[:], ps_oh[:])
                # rs = SCALE/sum(e) (ones=8 folded); broadcast via Pool
                rs = spool.tile([1, 512], FP32, tag="rs", bufs=1)
                nc.vector.reciprocal_approx_fast(out=rs[:], in_=ps_s[qch][:])
                bbS = spool.tile([128, 512], FP32, tag="bbS", bufs=1)
                nc.gpsimd.partition_broadcast(bbS[:], rs[:], channels=128)

                nc.vector.tensor_mul(ohr[hs, et2, qsl], ohraw[0:64, :], bbS[0:64, :])
                nc.vector.tensor_mul(ohi[hs, et2, qsl], ohraw[64:128, :], bbS[64:128, :])

        # software-pipelined drive: keep stage 1 ~1 e-tile ahead of heads
        stage1_et(0)
        stage1_et(1)
        head(0); head(1)
        stage1_et(2)
        head(2); head(3)
        stage1_et(3)
        head(4); head(5); head(6); head(7)

        # ---- stage 4: out_partial = oh @ WoutT[my cols]; AllReduce over pair ----
        for dc in range(2):
            dsl = slice(dc * 512, (dc + 1) * 512)
            wos = wopool.tile([128, 3 * ETL, 512], BF16, tag="wos")
            nc.sync.dma_start(out=wos[:], in_=wo_d.ap()[:, dsl].rearrange("(t p) n -> p t n", p=128))
            ob = drpool.tile([N, 1024], BF16, tag="ob")
            obv = ob.rearrange("n (c d) -> n c d", c=2)
            for qt in range(8):
                qsl = slice(qt * 128, (qt + 1) * 128)
                po = psd.tile([128, 2, 512], FP32, tag="dots")
                for et in range(ETL):
                    first = et == 0
                    # out_re = ohr.wr + ohi.(-wi) ; out_im = ohi.wr + ohr.wi
                    nc.tensor.matmul(po[:, 0, :], ohr[:, et, qsl], wos[:, et, :],
                                     start=first, stop=False)
                    nc.tensor.matmul(po[:, 0, :], ohi[:, et, qsl], wos[:, 2 * ETL + et, :],
                                     start=False, stop=(et == ETL - 1))
                    nc.tensor.matmul(po[:, 1, :], ohi[:, et, qsl], wos[:, et, :],
                                     start=first, stop=False)
                    nc.tensor.matmul(po[:, 1, :], ohr[:, et, qsl], wos[:, ETL + et, :],
                                     start=False, stop=(et == ETL - 1))
                o_st = opool.tile([128, 2, 512], BF16, tag="ost", bufs=1)
                nc.scalar.copy(o_st[:], po[:])
                nc.sync.dma_start(out=obv[qsl, :, :], in_=o_st[:])
            out_d = o0_d if dc == 0 else o1_d
            obr = drpool.tile([N, 1024], BF16, tag="obr")
            nc.gpsimd.collective_compute(
                "AllReduce", ALU.add, replica_groups=PAIRS,
                ins=[ob.opt()], outs=[obr.opt()],
            )
            nc.sync.dma_start(out=out_d.ap(), in_=obr[:])

    nc.compile()
    return nc


def _to_bf16(a):
    return np.asarray(a, dtype=np.float32).astype(ml_dtypes.bfloat16)


def make_in_maps(x_re, x_im, wqkv_re, wqkv_im, wout_re, wout_im, bout_re, bout_im):
    x_re = np.asarray(x_re, np.float32)
    x_im = np.asarray(x_im, np.float32)
    wqT_r = np.asarray(wqkv_re, np.float32).T   # [D, E]
    wqT_i = np.asarray(wqkv_im, np.float32).T
    woT_r = np.asarray(wout_re, np.float32).T   # [E, D]
    woT_i = np.asarray(wout_im, np.float32).T

    in_maps = []
    for c in range(8):
        b, g = c // 2, c % 2
        esl = slice(g * EL, (g + 1) * EL)
        in_maps.append({
            "xTr": _to_bf16(np.ascontiguousarray(x_re[b].T)),
            "xTi": _to_bf16(np.ascontiguousarray(x_im[b].T)),
            "wqTr": _to_bf16(np.ascontiguousarray(wqT_r[:, esl])),
            "wqTi": _to_bf16(np.ascontiguousarray(wqT_i[:, esl])),
            "wqTin": _to_bf16(np.ascontiguousarray(-wqT_i[:, esl])),
            "woS": _to_bf16(np.concatenate([woT_r[esl, :], woT_i[esl, :], -woT_i[esl, :]], axis=0)),
        })
    return in_maps


def assemble_output(res, bout_re, bout_im):
    out = np.zeros((B, N, D), np.complex64)
    for b in range(B):
        o0 = np.asarray(res[2 * b]["o0"], np.float32)
        o1 = np.asarray(res[2 * b]["o1"], np.float32)
        out[b, :, 0:512] = o0[:, 0:512] + 1j * o0[:, 512:1024]
        out[b, :, 512:1024] = o1[:, 0:512] + 1j * o1[:, 512:1024]

    # bout is zero in this problem; add anyway for faithfulness
    out += (np.asarray(bout_re, np.float32) + 1j * np.asarray(bout_im, np.float32))[None, None, :]
    return out


def kernel(**inputs):
    if "nc" not in _CACHE:
        _CACHE["nc"] = build_graph()
    nc = _CACHE["nc"]
    in_maps = make_in_maps(**inputs)
    res = run_bass_kernel_spmd(nc, in_maps, core_ids=list(range(8))).results
    return assemble_output(res, inputs["bout_re"], inputs["bout_im"])


# revision 34
# speedup vs baseline: 1.2013x; 1.0663x over previous
"""Complex-valued attention (magnitude-softmax with phase reconstruction) on 8 TRN2 cores.

Sharding: core c -> (batch b = c//2, head-group g = c%2). Each core projects
only its 8 heads (wqkv columns host-sliced per core), runs the attention loop
for those heads over ALL 1024 queries, and computes a partial output
contraction over its 512 e-columns of Wout. A pairwise AllReduce(add) over
{2b, 2b+1} then produces the full [N, D] output on both cores (assembled from
the even cores host-side). No token permutation is needed anywhere.

Math (per head h, scale S = 1/8):
  w = x @ Wqkv^T                         (bf16 matmuls, fp32 accum)
  z = dots^T[k, q]                       computed TRANSPOSED via stacked
      re/im contraction so softmax ends up on the partition axis
  m2 = z_re^2 + z_im^2                   (custom fused DVE op, bf16 inputs)
  l = ln(m2+eps);  e1 = exp(0.5*l + ln(S)) = S*mag   (affine fused into ACT)
  e = exp(e1)  (softmax numerator; mags are bounded so no max-subtraction)
  rm = exp(-0.5*l - ln(S)) = 1/(S*mag)   (ACT, same table)
  f = e * rm;  attnU = z * f  (= 8 * e * unit_phase)
  oh^T = sum_k wcombo^T @ attnU^T ; scaled by SCALE/sum_k e via ones=8.0
      S-matmul + reciprocal + Pool partition_broadcast
  out_partial = oh @ WoutT[my cols] ; AllReduce(add) over the core pair

Engine balance per kp (2 key tiles x 512 queries):
  PE:   4 dots mm + 4 oh mm + 2 S mm (+ stage-1/stage-4 interleaved)
  Pool: zre/zim PSUM->SBUF bf16 evictions + per-(h,qch) partition_broadcast
  ACT:  Ln, Exp(e1), Exp(ee), Exp(rm) (one table set: natural_log_exp_and_others)
  DVE:  sqmag, f-mul, attnU re/im muls (bf16 2x mode)
"""

import os
import sys
import numpy as np
import ml_dtypes

sys.path.insert(0, "/opt/trn_rl_repo")

from contextlib import ExitStack

import concourse.bass as bass
import concourse.tile as tile
from concourse import bacc, mybir, dve_ops
from concourse.bass_utils import run_bass_kernel_spmd
from concourse.dve_spec import Spec, Src0, Src1, sq, lower, _has_src1
from concourse.dve_uop import DveOpSpec


def _register_sqmag():
    """Custom DVE op: out = Src0^2 + Src1^2 (one instr instead of 2 ACT + 1 DVE)."""
    name = "TENSOR_SQMAG_ANT"
    if name in dve_ops._SUB_OPCODE_FOR_NAME:
        return next(o for o in dve_ops.OPS if o.name == name)
    spec = Spec(
        body=sq(Src0) + sq(Src1),
        reference=lambda in0, in1, s0, s1, imm2:
            (in0.astype(np.float32) ** 2 + in1.astype(np.float32) ** 2),
    )
    opcode = max(dve_ops._SUB_OPCODE_FOR_NAME.values()) + 1
    dve_ops._SUB_OPCODE_FOR_NAME[name] = opcode
    shas = {}
    for ver in ("v3", "v4"):
        uops = lower(spec, ver=ver)
        shas[ver] = DveOpSpec(name=name, opcode=opcode, uops=uops,
                              rd1_en=_has_src1(spec)).sha(ver)
    op = dve_ops.DveOp(name, spec, subdim=False, uops_sha=shas)
    dve_ops.OPS.append(op)
    dve_ops.CUSTOM_DVE_SPECS[name] = spec
    return op


SQMAG = _register_sqmag()


def _patch_act_tables():
    """Force exp/ln to resolve to the combined natural_log_exp_and_others set
    so the per-tile Ln->Exp alternation doesn't reload ACT tables (~2.7us each)."""
    import concourse.bacc as _bacc
    if getattr(_bacc, "_act_tables_patched", False):
        return
    orig = _bacc.get_activation_tables
    AFT = mybir.ActivationFunctionType

    def patched(arch):
        t = {k: set(v) for k, v in orig(arch).items()}
        for name, fns in t.items():
            if name != "natural_log_exp_and_others":
                fns.discard(AFT.Exp)
                fns.discard(AFT.Ln)
        return t

    _bacc.get_activation_tables = patched
    _bacc._act_tables_patched = True


_patch_act_tables()

B, N, D, H, DH = 4, 1024, 1024, 16, 64
E = H * DH          # 1024
HL = 8              # heads per core
EL = 512            # local e-columns (HL * DH)
NQ = N              # queries per core: all 1024
KT = 8              # key tiles of 128
DT = 8              # d (contraction) tiles of 128
ETL = 4             # local e tiles of 128 (2 heads each)
SCALE = DH ** -0.5  # 0.125
LN_S = float(np.log(SCALE))
EPS = 1e-20

FP32 = mybir.dt.float32
BF16 = mybir.dt.bfloat16
AF = mybir.ActivationFunctionType
ALU = mybir.AluOpType

PAIRS = [[0, 1], [2, 3], [4, 5], [6, 7]]

_CACHE = {}


def build_graph():
    nc = bacc.Bacc("TRN2", target_bir_lowering=False, debug=False,
                   enable_asserts=False, num_devices=8)

    xTr_d = nc.declare_dram_parameter("xTr", [D, N], BF16, isOutput=False)
    xTi_d = nc.declare_dram_parameter("xTi", [D, N], BF16, isOutput=False)
    wqr_d = nc.declare_dram_parameter("wqTr", [D, EL], BF16, isOutput=False)   # my Wqkv_re.T cols
    wqi_d = nc.declare_dram_parameter("wqTi", [D, EL], BF16, isOutput=False)   # my Wqkv_im.T cols
    wqin_d = nc.declare_dram_parameter("wqTin", [D, EL], BF16, isOutput=False)  # -my Wqkv_im.T cols
    wo_d = nc.declare_dram_parameter("woS", [3 * EL, D], BF16, isOutput=False)  # [Wout_re.T; Wout_im.T; -Wout_im.T] my rows
    # outputs: per d-half chunk, rows = all N queries, cols = [re 512 | im 512]
    # bf16: halves the AllReduce + copy volume in the output tail
    o0_d = nc.declare_dram_parameter("o0", [N, 1024], BF16, isOutput=True)
    o1_d = nc.declare_dram_parameter("o1", [N, 1024], BF16, isOutput=True)

    with tile.TileContext(nc) as tc, ExitStack() as ctx:
        const_pool = ctx.enter_context(tc.tile_pool(name="const", bufs=1))
        xpool = ctx.enter_context(tc.tile_pool(name="x", bufs=1))
        wqpool = ctx.enter_context(tc.tile_pool(name="wq", bufs=1))
        apool = ctx.enter_context(tc.tile_pool(name="A", bufs=1))
        bpool = ctx.enter_context(tc.tile_pool(name="B", bufs=2))
        wcpool = ctx.enter_context(tc.tile_pool(name="wc", bufs=2))
        ohpool = ctx.enter_context(tc.tile_pool(name="oh", bufs=1))
        wopool = ctx.enter_context(tc.tile_pool(name="wo", bufs=1))
        epool = ctx.enter_context(tc.tile_pool(name="elem", bufs=2))
        spool = ctx.enter_context(tc.tile_pool(name="sm", bufs=2))
        opool = ctx.enter_context(tc.tile_pool(name="ostage", bufs=1))
        stkpool = ctx.enter_context(tc.tile_pool(name="stk", bufs=1))
        psd = ctx.enter_context(tc.tile_pool(name="psd", bufs=2, space="PSUM"))
        psd1 = ctx.enter_context(tc.tile_pool(name="psd1", bufs=1, space="PSUM"))
        psoh = ctx.enter_context(tc.tile_pool(name="psoh", bufs=1, space="PSUM"))
        psS = ctx.enter_context(tc.tile_pool(name="psS", bufs=2, space="PSUM"))
        drpool = ctx.enter_context(tc.tile_pool(name="dram", bufs=2, space="DRAM"))

        # ---- constants ----
        # ones = 8.0 folds the SCALE factor into the softmax-sum reciprocal:
        # rs = 1/(8*sum e) = SCALE/sum e
        ones_bf = const_pool.tile([128, 1], BF16, tag="ones")
        nc.gpsimd.memset(ones_bf[:], 1.0 / SCALE)
        eps_t = const_pool.tile([128, 1], FP32, tag="eps")
        nc.gpsimd.memset(eps_t[:], EPS)
        lnS_t = const_pool.tile([128, 1], FP32, tag="lnS")
        nc.gpsimd.memset(lnS_t[:], LN_S)
        nlnS_t = const_pool.tile([128, 1], FP32, tag="nlnS")
        nc.gpsimd.memset(nlnS_t[:], -LN_S)

        # ---- resident loads ----
        # x^T as [128, DT, N] (partition = d within tile)
        xr = xpool.tile([128, DT, N], BF16, tag="xr")
        xi = xpool.tile([128, DT, N], BF16, tag="xi")
        nc.sync.dma_start(out=xr[:], in_=xTr_d.ap().rearrange("(t p) n -> p t n", p=128))
        nc.sync.dma_start(out=xi[:], in_=xTi_d.ap().rearrange("(t p) n -> p t n", p=128))
        # A stacks: per head [128 (wr 0:64 | wi 64:128), N]; ring of 6 slots
        A = []

        # ---- stage 1: w^T = Wqkv~ @ x^T (my 8 heads), evicted to per-head stacks ----
        def stage1_et(et):
            A.append(apool.tile([128, N], BF16, tag="A", name=f"A{2*et}", bufs=4))
            A.append(apool.tile([128, N], BF16, tag="A", name=f"A{2*et+1}", bufs=4))
            wslab_r = wqpool.tile([128, DT, 128], BF16, tag="wslab_r")
            wslab_i = wqpool.tile([128, DT, 128], BF16, tag="wslab_i")
            wslab_in = wqpool.tile([128, DT, 128], BF16, tag="wslab_in")
            esl = slice(et * 128, (et + 1) * 128)
            nc.sync.dma_start(out=wslab_r[:], in_=wqr_d.ap()[:, esl].rearrange("(t p) n -> p t n", p=128))
            nc.sync.dma_start(out=wslab_i[:], in_=wqi_d.ap()[:, esl].rearrange("(t p) n -> p t n", p=128))
            nc.sync.dma_start(out=wslab_in[:], in_=wqin_d.ap()[:, esl].rearrange("(t p) n -> p t n", p=128))
            h0, h1 = 2 * et, 2 * et + 1
            Ar = A[h0].rearrange("p (c n) -> p c n", c=2)
            Ai = A[h1].rearrange("p (c n) -> p c n", c=2)
            # per-nch PSUM tiles: holds only 2 banks at a time (vs 4) so the
            # head-loop dots keep a double-buffer, and heads can start on the
            # first token half before the second is projected
            for nch in range(2):
                nsl = slice(nch * 512, (nch + 1) * 512)
                ps_re = psd1.tile([128, 512], FP32, tag="s1")
                ps_im = psd1.tile([128, 512], FP32, tag="s1")
                for dt_ in range(DT):
                    first, last = dt_ == 0, dt_ == DT - 1
                    # w_re += Wr^T x_r ; w_re += (-Wi^T) x_i
                    nc.tensor.matmul(ps_re[:], wslab_r[:, dt_, :], xr[:, dt_, nsl],
                                     start=first, stop=False)
                    nc.tensor.matmul(ps_re[:], wslab_in[:, dt_, :], xi[:, dt_, nsl],
                                     start=False, stop=last)
                    # w_im += Wi^T x_r ; w_im += Wr^T x_i
                    nc.tensor.matmul(ps_im[:], wslab_i[:, dt_, :], xr[:, dt_, nsl],
                                     start=first, stop=False)
                    nc.tensor.matmul(ps_im[:], wslab_r[:, dt_, :], xi[:, dt_, nsl],
                                     start=False, stop=last)
                # evict into head stacks (cast to bf16)
                nc.scalar.copy(Ar[0:64, nch, :], ps_re[0:64, :])
                nc.scalar.copy(Ai[0:64, nch, :], ps_re[64:128, :])
                nc.scalar.copy(Ar[64:128, nch, :], ps_im[0:64, :])
                nc.scalar.copy(Ai[64:128, nch, :], ps_im[64:128, :])

        # ---- per-head attention (Hermitian: only lower-triangle pair-strips) ----
        # oh^T stacks for stage 4: [128, ETL, NQ] bf16 (ohin folded into wos)
        ohr = ohpool.tile([128, ETL, NQ], BF16, tag="ohr")
        ohi = ohpool.tile([128, ETL, NQ], BF16, tag="ohi")

        # pair p covers k-tiles {2p, 2p+1} and padded q-cols [0, (2p+2)*128),
        # processed in <=512-wide chunks; the upper triangle is reconstructed by
        # DMA-transposing each row's strictly-lower-pair cols [0, 2p*128).
        PAIR_CHUNKS = [
            [(0, 256)],
            [(0, 512)],
            [(0, 512), (512, 256)],
            [(0, 512), (512, 512)],
        ]

        def head(h):
            Ah = A[h]
            # B_h = [-wi; wr]; Bhn = -B_h (stationary for transposed-aim pieces)
            Bh = bpool.tile([128, N], BF16, tag="B", bufs=1)
            nc.vector.tensor_scalar_mul(Bh[0:64, :], Ah[64:128, :], -1.0)
            nc.vector.tensor_copy(Bh[64:128, :], Ah[0:64, :])
            Bhn = bpool.tile([128, N], BF16, tag="Bn", bufs=1)
            nc.vector.tensor_scalar_mul(Bhn[:], Bh[:], -1.0)

            wc1 = wcpool.tile([128, KT, 128], BF16, tag="wc1")
            wc2 = wcpool.tile([128, KT, 128], BF16, tag="wc2")
            wc2n = wcpool.tile([128, KT, 128], BF16, tag="wc2n")
            nc.sync.dma_start(wc1[:], Ah[:], transpose=True)
            nc.sync.dma_start(wc2[:], Bh[:], transpose=True)
            nc.sync.dma_start(wc2n[:], Bhn[:], transpose=True)

            areF = stkpool.tile([128, KT, N], BF16, tag="areF", bufs=2)
            aimF = stkpool.tile([128, KT, N], BF16, tag="aimF", bufs=2)
            eeF = stkpool.tile([128, KT, N], BF16, tag="eeF", bufs=1)

            # pair order (1,2,3,0): start on a pair whose dots need only the
            # first token-half of A (pipeline-friendly head start), and end on
            # pair 0 (no transposes, smallest chunk) so pair 3's wide XBAR
            # transposes drain during pair 0's compute instead of serializing
            # the end-of-head S-matmuls and the next head's eeF reuse.
            for p in (1, 2, 3, 0):
                r2 = slice(2 * p, 2 * p + 2)
                for (qoff, w) in PAIR_CHUNKS[p]:
                    qws = slice(qoff, qoff + w)
                    zre = psd.tile([128, 2, 512], FP32, tag="dots")
                    zim = psd.tile([128, 2, 512], FP32, tag="dots")
                    # dots^T[k, q]: re = [wr;wi]_k . [wr;wi]_q ; im = [-wi;wr]_k . [wr;wi]_q
                    for i in range(2):
                        ksl = slice((2 * p + i) * 128, (2 * p + i + 1) * 128)
                        nc.tensor.matmul(zre[:, i, 0:w], Ah[:, ksl], Ah[:, qws],
                                         start=True, stop=True)
                        nc.tensor.matmul(zim[:, i, 0:w], Bh[:, ksl], Ah[:, qws],
                                         start=True, stop=True)

                    # evict both dots tensors as bf16: PSUM recycles fast and the
                    # tail multiplies run in DVE 2x mode
                    zreS = epool.tile([128, 2, 512], BF16, tag="zreS", bufs=2)
                    nc.scalar.copy(zreS[:, :, 0:w], zre[:, :, 0:w])
                    zimS = epool.tile([128, 2, 512], BF16, tag="zimS", bufs=2)
                    nc.scalar.copy(zimS[:, :, 0:w], zim[:, :, 0:w])
                    m2 = epool.tile([128, 2, 512], FP32, tag="m2", bufs=2)
                    nc.vector._custom_dve(SQMAG, out=m2[:, :, 0:w],
                                          in0=zreS[:, :, 0:w], in1=zimS[:, :, 0:w])
                    # ll = ln(m2+eps) computed in place over m2 (saves a tag)
                    nc.scalar.activation(m2[:, :, 0:w], m2[:, :, 0:w], AF.Ln, bias=eps_t[:])
                    # e1 = S*mag = exp(0.5*ln(m2) + ln(S)) -- affine fused into the ACT op
                    e1 = epool.tile([128, 2, 512], FP32, tag="e1", bufs=2)
                    nc.scalar.activation(e1[:, :, 0:w], m2[:, :, 0:w], AF.Exp,
                                         scale=0.5, bias=lnS_t[:])
                    nc.scalar.activation(eeF[:, r2, qws], e1[:, :, 0:w], AF.Exp)
                    # rm = 1/e1 in place over e1
                    nc.vector.reciprocal_approx_fast(out=e1[:, :, 0:w], in_=e1[:, :, 0:w])
                    ff = epool.tile([128, 2, 512], BF16, tag="ff", bufs=1)
                    nc.vector.tensor_mul(ff[:, :, 0:w], eeF[:, r2, qws], e1[:, :, 0:w])
                    nc.vector.tensor_mul(areF[:, r2, qws], zreS[:, :, 0:w], ff[:, :, 0:w])
                    nc.vector.tensor_mul(aimF[:, r2, qws], zimS[:, :, 0:w], ff[:, :, 0:w])

                # reconstruct upper blocks: transpose each row's strictly-lower
                # pair cols [0, 2p*128) into column-tile kt of rows 0..2p-1
                if p >= 1:
                    src_cols = slice(0, 2 * p * 128)
                    for i in range(2):
                        kt = 2 * p + i
                        ktsl = slice(kt * 128, (kt + 1) * 128)
                        for F in (areF, aimF, eeF):
                            nc.sync.dma_start(F[:, 0:2 * p, ktsl], F[:, kt, src_cols],
                                              transpose=True)

            et2, half = h // 2, (h % 2) * 64
            hs = slice(half, half + 64)

            # softmax-sum matmuls first (releases eeF for the next head early)
            ps_s = []
            for qch in range(2):
                qsl = slice(qch * 512, (qch + 1) * 512)
                ps = psS.tile([1, 512], FP32, tag="S")
                for kt in range(KT):
                    nc.tensor.matmul(ps[:], ones_bf[:], eeF[:, kt, qsl],
                                     start=(kt == 0), stop=(kt == KT - 1))
                ps_s.append(ps)

            for qch in range(2):
                qoff = qch * 512
                qsl = slice(qoff, qoff + 512)
                ps_oh = psoh.tile([128, 512], FP32, tag="oh")
                for kt in range(KT):
                    first, last = kt == 0, kt == KT - 1
                    nc.tensor.matmul(ps_oh[:], wc1[:, kt, :], areF[:, kt, qsl],
                                     start=first, stop=False)
                    # aim sign: cols < pair-width are computed (true values, wc2);
                    # cols >= pair-width are transposed (negate via wc2n)
                    b = (2 * (kt // 2) + 2) * 128 - qoff
                    lo = max(0, min(512, b))
                    if lo > 0:
                        nc.tensor.matmul(ps_oh[:, 0:lo], wc2[:, kt, :],
                                         aimF[:, kt, qoff:qoff + lo],
                                         start=False, stop=last)
                    if lo < 512:
                        nc.tensor.matmul(ps_oh[:, lo:512], wc2n[:, kt, :],
                                         aimF[:, kt, qoff + lo:qoff + 512],
                                         start=False, stop=last)

                # evict raw oh^T immediately (releases psoh for the next chunk)
                ohraw = spool.tile([128, 512], FP32, tag="ohraw", bufs=2)
                nc.scalar.copy(ohraw[:], ps_oh[:])
                # rs = SCALE/sum(e) (ones=8 folded); broadcast via Pool
                rs = spool.tile([1, 512], FP32, tag="rs", bufs=1)
                nc.vector.reciprocal_approx_fast(out=rs[:], in_=ps_s[qch][:])
                bbS = spool.tile([128, 512], FP32, tag="bbS", bufs=1)
                nc.gpsimd.partition_broadcast(bbS[:], rs[:], channels=128)

                nc.vector.tensor_mul(ohr[hs, et2, qsl], ohraw[0:64, :], bbS[0:64, :])
                nc.vector.tensor_mul(ohi[hs, et2, qsl], ohraw[64:128, :], bbS[64:128, :])

        # software-pipelined drive: keep stage 1 ~1 e-tile ahead of heads
        stage1_et(0)
        stage1_et(1)
        head(0); head(1)
        stage1_et(2)
        head(2); head(3)
        stage1_et(3)
        head(4); head(5); head(6); head(7)

        # ---- stage 4: out_partial = oh @ WoutT[my cols]; AllReduce over pair ----
        for dc in range(2):
            dsl = slice(dc * 512, (dc + 1) * 512)
            wos = wopool.tile([128, 3 * ETL, 512], BF16, tag="wos")
            nc.sync.dma_start(out=wos[:], in_=wo_d.ap()[:, dsl].rearrange("(t p) n -> p t n", p=128))
            ob = drpool.tile([N, 1024], BF16, tag="ob")
            obv = ob.rearrange("n (c d) -> n c d", c=2)
            for qt in range(8):
                qsl = slice(qt * 128, (qt + 1) * 128)
                po = psd.tile([128, 2, 512], FP32, tag="dots")
                for et in range(ETL):
                    first = et == 0
                    # out_re = ohr.wr + ohi.(-wi) ; out_im = ohi.wr + ohr.wi
                    nc.tensor.matmul(po[:, 0, :], ohr[:, et, qsl], wos[:, et, :],
                                     start=first, stop=False)
                    nc.tensor.matmul(po[:, 0, :], ohi[:, et, qsl], wos[:, 2 * ETL + et, :],
                                     start=False, stop=(et == ETL - 1))
                    nc.tensor.matmul(po[:, 1, :], ohi[:, et, qsl], wos[:, et, :],
                                     start=first, stop=False)
                    nc.tensor.matmul(po[:, 1, :], ohr[:, et, qsl], wos[:, ETL + et, :],
                                     start=False, stop=(et == ETL - 1))
                o_st = opool.tile([128, 2, 512], BF16, tag="ost", bufs=1)
                nc.scalar.copy(o_st[:], po[:])
                nc.sync.dma_start(out=obv[qsl, :, :], in_=o_st[:])
            out_d = o0_d if dc == 0 else o1_d
            obr = drpool.tile([N, 1024], BF16, tag="obr")
            nc.gpsimd.collective_compute(
                "AllReduce", ALU.add, replica_groups=PAIRS,
                ins=[ob.opt()], outs=[obr.opt()],
            )
            nc.sync.dma_start(out=out_d.ap(), in_=obr[:])

    nc.compile()
    return nc


def _to_bf16(a):
    return np.asarray(a, dtype=np.float32).astype(ml_dtypes.bfloat16)


def make_in_maps(x_re, x_im, wqkv_re, wqkv_im, wout_re, wout_im, bout_re, bout_im):
    x_re = np.asarray(x_re, np.float32)
    x_im = np.asarray(x_im, np.float32)
    wqT_r = np.asarray(wqkv_re, np.float32).T   # [D, E]
    wqT_i = np.asarray(wqkv_im, np.float32).T
    woT_r = np.asarray(wout_re, np.float32).T   # [E, D]
    woT_i = np.asarray(wout_im, np.float32).T

    in_maps = []
    for c in range(8):
        b, g = c // 2, c % 2
        esl = slice(g * EL, (g + 1) * EL)
        in_maps.append({
            "xTr": _to_bf16(np.ascontiguousarray(x_re[b].T)),
            "xTi": _to_bf16(np.ascontiguousarray(x_im[b].T)),
            "wqTr": _to_bf16(np.ascontiguousarray(wqT_r[:, esl])),
            "wqTi": _to_bf16(np.ascontiguousarray(wqT_i[:, esl])),
            "wqTin": _to_bf16(np.ascontiguousarray(-wqT_i[:, esl])),
            "woS": _to_bf16(np.concatenate([woT_r[esl, :], woT_i[esl, :], -woT_i[esl, :]], axis=0)),
        })
    return in_maps


def assemble_output(res, bout_re, bout_im):
    out = np.zeros((B, N, D), np.complex64)
    for b in range(B):
        o0 = np.asarray(res[2 * b]["o0"], np.float32)
        o1 = np.asarray(res[2 * b]["o1"], np.float32)
        out[b, :, 0:512] = o0[:, 0:512] + 1j * o0[:, 512:1024]
        out[b, :, 512:1024] = o1[:, 0:512] + 1j * o1[:, 512:1024]

    # bout is zero in this problem; add anyway for faithfulness
    out += (np.asarray(bout_re, np.float32) + 1j * np.asarray(bout_im, np.float32))[None, None, :]
    return out


def kernel(**inputs):
    if "nc" not in _CACHE:
        _CACHE["nc"] = build_graph()
    nc = _CACHE["nc"]
    in_maps = make_in_maps(**inputs)
    res = run_bass_kernel_spmd(nc, in_maps, core_ids=list(range(8))).results
    return assemble_output(res, inputs["bout_re"], inputs["bout_im"])
